# revision 1
# baseline (speedup 1.0000x reference)
"""AttentionPool TRN2 kernel.

Problem: B=2048, S=512, D=128, H=4, T=8 (Q = T*H = 32), C=64.
  k = keys @ Wk^T ; v = keys @ Wv^T
  q = q_flat + (ctx @ Wc^T + bc).reshape(B, Q, D)
  attn = (q @ k^T) * scale * inv_t[q] - slopes[q] * games_ago[s]
  out  = softmax_masked(attn) @ v            -> [B, T, H*D]

Restructured so `keys` is touched by exactly two matmuls per row:
  logits[q,s] = qk'[q,:]  . keys[s,:]        (qk' = (q @ Wk) * scale*inv_t, host-folded)
  pooled[q,:] = (w[q,:] @ keys) @ Wv^T
ALiBi: on unmasked positions games_ago = n_real-1-s, so
  -slope*(n-1-s) = slope*s - slope*(n-1): a batch-independent [Q,S] tile
  plus a per-row constant, which we also use as the softmax shift.

Sharding: pure data parallel over batch, 256 rows/core on 8 cores.
"""

import sys

if "/opt/trn_rl_repo" not in sys.path:
    sys.path.insert(0, "/opt/trn_rl_repo")

import numpy as np

import concourse.bacc as bacc
import concourse.bass as bass
import concourse.tile as tile
from concourse import mybir
from concourse.bass_utils import run_bass_kernel_spmd

B, S, D, H, T, C = 2048, 512, 128, 4, 8, 64
Q = T * H  # 32
N_CORES = 8
ROWS = B // N_CORES  # 256 rows per core
GRP = 4  # batch rows per group -> 4*32 = 128 partitions
BLK = 128  # rows per block (mask/ctx/QKT staging)
SC = 64.0  # power-of-two prescale keeping fp16 operands in normal range
MASK_NEG = 16384.0  # fp16/f32-exact; /SC = 256 pushes masked logits below -126

F32 = mybir.dt.float32
F16 = mybir.dt.float16
U8 = mybir.dt.uint8


def _emit(nc, tc, rows):
    """Emit the per-core program for `rows` batch rows (rows % GRP == 0)."""
    keys_d = nc.declare_dram_parameter("keys", [rows, S, D], F32, isOutput=False)
    mask_d = nc.declare_dram_parameter("mask", [rows, S], U8, isOutput=False)
    ctx_d = nc.declare_dram_parameter("ctx", [rows, C], F32, isOutput=False)
    maug_d = nc.declare_dram_parameter("maug", [C + 1, Q, D], F16, isOutput=False)
    wvt_d = nc.declare_dram_parameter("wvt", [D, D], F32, isOutput=False)
    sbias_d = nc.declare_dram_parameter("sbias", [128, S], F32, isOutput=False)
    mstat_d = nc.declare_dram_parameter("mstat", [GRP, 128], F16, isOutput=False)
    id16_d = nc.declare_dram_parameter("id16", [128, 128], F16, isOutput=False)
    id32_d = nc.declare_dram_parameter("id32", [128, 128], F32, isOutput=False)
    out_d = nc.declare_dram_parameter("out", [rows, Q * D], F32, isOutput=True)

    keys_ap = keys_d.ap()
    mask_ap = mask_d.ap()
    ctx_ap = ctx_d.ap()
    out_ap = out_d.ap()

    n_blk = (rows + BLK - 1) // BLK

    import contextlib

    with contextlib.ExitStack() as ctx:
        singles = ctx.enter_context(tc.tile_pool(name="singles", bufs=1))
        kpool = ctx.enter_context(tc.tile_pool(name="kpool", bufs=6))
        ktpool = ctx.enter_context(tc.tile_pool(name="ktpool", bufs=4))
        blkpool = ctx.enter_context(tc.tile_pool(name="blkpool", bufs=2))
        qktpool = ctx.enter_context(tc.tile_pool(name="qktpool", bufs=2))
        work = ctx.enter_context(tc.tile_pool(name="work", bufs=3))
        small = ctx.enter_context(tc.tile_pool(name="small", bufs=4))
        ps = ctx.enter_context(tc.tile_pool(name="ps", bufs=1, space="PSUM"))

        # ---- constants (loaded once) ----
        maug_sb = singles.tile([C + 1, Q, D], F16)
        nc.sync.dma_start(out=maug_sb, in_=maug_d.ap())
        wvt_sb = singles.tile([D, D], F32)
        nc.sync.dma_start(out=wvt_sb, in_=wvt_d.ap())
        sbias_sb = singles.tile([128, S], F32)
        nc.sync.dma_start(out=sbias_sb, in_=sbias_d.ap())
        mstat_sb = singles.tile([GRP, 128], F16)
        nc.sync.dma_start(out=mstat_sb, in_=mstat_d.ap())
        id16_sb = singles.tile([128, 128], F16)
        nc.sync.dma_start(out=id16_sb, in_=id16_d.ap())
        id32_sb = singles.tile([128, 128], F32)
        nc.sync.dma_start(out=id32_sb, in_=id32_d.ap())

        # ---- prologue: conditioned queries qk'^T for every block ----
        qkt_blocks = []
        for blk in range(n_blk):
            r0 = blk * BLK
            bn = min(BLK, rows - r0)
            assert bn % GRP == 0

            ctx_sb = blkpool.tile([BLK, C], F32, tag="ctx")
            nc.sync.dma_start(out=ctx_sb[:bn], in_=ctx_ap[r0 : r0 + bn])
            ctxt_ps = ps.tile([C, BLK], F32, tag="smallf32", bufs=2)
            nc.tensor.transpose(ctxt_ps[:, :bn], ctx_sb[:bn], id32_sb[:bn, :bn])
            ctxt_sb = blkpool.tile([C + 1, BLK], F16, tag="ctxt")
            nc.vector.tensor_copy(out=ctxt_sb[:C, :bn], in_=ctxt_ps[:, :bn])
            nc.vector.memset(ctxt_sb[C : C + 1, :bn], 1.0)

            # qk'^T for the block: [D, bn, Q] fp16 (prescaled by SC*scale*inv_t)
            qkt_sb = qktpool.tile([D, BLK, Q], F16, tag="qkt")
            for q in range(Q):
                qkt_ps = ps.tile([D, BLK], F32, tag="smallf32", bufs=2)
                nc.tensor.matmul(
                    qkt_ps[:, :bn], maug_sb[:, q, :], ctxt_sb[:, :bn],
                    start=True, stop=True,
                )
                nc.vector.tensor_copy(out=qkt_sb[:, :bn, q], in_=qkt_ps[:, :bn])
            qkt_blocks.append(qkt_sb)

        n_grp_total = rows // GRP
        PF = 3  # software prefetch distance (groups)
        staged = {}

        def _load_group(g):
            if g >= n_grp_total or g in staged:
                return
            g0 = g * GRP
            k32 = kpool.tile([128, GRP, S // 128, D], F32, tag="k32",
                             name=f"k32_{g}")
            for r in range(GRP):
                nc.sync.dma_start(
                    out=k32[:, r],
                    in_=keys_ap[g0 + r].rearrange("(c p) d -> p c d", p=128),
                )
            k16 = kpool.tile([128, GRP, S // 128, D], F16, tag="k16",
                             name=f"k16_{g}")
            nc.gpsimd.tensor_copy(out=k16, in_=k32)
            masku_g = small.tile([GRP, S], U8, tag="masku", name=f"masku_{g}", bufs=6)
            nc.sync.dma_start(out=masku_g, in_=mask_ap[g0 : g0 + GRP])
            maskp_g = small.tile([GRP, S], F16, tag="maskp", name=f"maskp_{g}", bufs=6)
            nc.gpsimd.tensor_copy(out=maskp_g, in_=masku_g)
            staged[g] = (k16, maskp_g)

        for g in range(PF):
            _load_group(g)

        for g in range(n_grp_total):
            g0 = g * GRP  # absolute row of this group
            qkt_sb = qkt_blocks[g0 // BLK]
            gl = (g0 % BLK) // GRP
            _load_group(g + PF)
            if True:
                k16, maskp_g = staged.pop(g)

                # ---- keys^T per row: [d, s] via PE transpose ----
                kt_sb = ktpool.tile([128, GRP, S], F16, tag="kt")
                for r in range(GRP):
                    ktp = ps.tile([128, S], F16, tag="ktp", bufs=3)
                    for c in range(S // 128):
                        nc.tensor.transpose(
                            ktp[:, c * 128 : (c + 1) * 128], k16[:, r, c, :], id16_sb
                        )
                    nc.scalar.copy(out=kt_sb[:, r, :], in_=ktp)

                # ---- pass 1: logits = qk'.keys + SC*slope*s + mask ----
                lg_ps = ps.tile([128, S], F32, tag="logits", bufs=2)
                for r in range(GRP):
                    nc.tensor.matmul(
                        lg_ps[32 * r : 32 * (r + 1), :],
                        qkt_sb[:, gl * GRP + r, :],
                        kt_sb[:, r, :],
                        start=True, stop=False,
                        tile_position=(0, 32 * r),
                        skip_group_check=True,
                    )
                nc.tensor.matmul(
                    lg_ps, mstat_sb, maskp_g,
                    start=False, stop=True,
                    skip_group_check=True,
                )

                tmp_sb = work.tile([128, S], F32, tag="tmp")
                nc.vector.tensor_add(tmp_sb, lg_ps, sbias_sb)

                # softmax shift = true row max (negated, prescaled for ACT bias)
                mx_sb = small.tile([128, 1], F32, tag="mx")
                nc.vector.tensor_reduce(
                    out=mx_sb, in_=tmp_sb, axis=mybir.AxisListType.X,
                    op=mybir.AluOpType.max,
                )
                cb_sb = small.tile([128, 1], F32, tag="cb")
                nc.vector.tensor_scalar(
                    out=cb_sb, in0=mx_sb, scalar1=-1.0 / SC, scalar2=None,
                    op0=mybir.AluOpType.mult,
                )

                e_sb = work.tile([128, S], F32, tag="e")
                sum_sb = small.tile([128, 1], F32, tag="sum")
                nc.scalar.activation(
                    out=e_sb, in_=tmp_sb, func=mybir.ActivationFunctionType.Exp,
                    bias=cb_sb, scale=1.0 / SC, accum_out=sum_sb,
                )
                rs_sb = small.tile([128, 1], F32, tag="rs")
                nc.vector.reciprocal(rs_sb, sum_sb)

                # ---- w^T: [s_in_chunk, c, rq] fp16 ----
                wt_ps = ps.tile([128, S // 128, 128], F32, tag="wtps", bufs=1)
                for c in range(S // 128):
                    nc.tensor.transpose(
                        wt_ps[:, c, :], e_sb[:, c * 128 : (c + 1) * 128], id32_sb
                    )
                wt_sb = work.tile([128, S // 128, 128], F16, tag="wt")
                nc.vector.tensor_copy(out=wt_sb, in_=wt_ps)

                # ---- pass 2: pk^T[d, rq] = sum_s keys[s,d] * w[rq,s] ----
                pk_ps = ps.tile([128, 128], F32, tag="smallf32", bufs=2)
                for r in range(GRP):
                    for c in range(S // 128):
                        nc.tensor.matmul(
                            pk_ps[:, 32 * r : 32 * (r + 1)],
                            k16[:, r, c, :],
                            wt_sb[:, c, 32 * r : 32 * (r + 1)],
                            start=(c == 0), stop=(c == S // 128 - 1),
                            skip_group_check=True,
                        )
                pkt_sb = work.tile([128, 128], F32, tag="pkt")
                nc.vector.tensor_copy(out=pkt_sb, in_=pk_ps)

                # ---- pooled[rq, e] = pk^T.T @ Wv^T  (exact fp32) ----
                po_ps = ps.tile([128, 128], F32, tag="smallf32", bufs=2)
                nc.tensor.matmul(po_ps, pkt_sb, wvt_sb, start=True, stop=True)

                o_sb = work.tile([128, 128], F32, tag="o")
                nc.vector.tensor_scalar(
                    out=o_sb, in0=po_ps, scalar1=rs_sb, scalar2=None,
                    op0=mybir.AluOpType.mult,
                )
                nc.scalar.dma_start(
                    out=out_ap[g0 : g0 + GRP].rearrange("r (q e) -> (r q) e", e=D),
                    in_=o_sb,
                )


def _build(rows):
    nc = bacc.Bacc(trn_type="TRN2", target_bir_lowering=False, debug=False)
    with tile.TileContext(nc) as tc:
        _emit(nc, tc, rows)
    nc.compile()
    return nc


def host_consts(queries, Wk, log_temperature, Wc, bc, Wv):
    """Fold projections/scales into small host-side constants."""
    queries = np.asarray(queries, np.float64)
    Wk = np.asarray(Wk, np.float64)
    Wc = np.asarray(Wc, np.float64)
    bc = np.asarray(bc, np.float64)
    Wv = np.asarray(Wv, np.float64)
    lt = np.asarray(log_temperature, np.float64)

    scale = D ** -0.5
    inv_t = np.repeat(np.exp(-lt), H)  # [Q]
    slopes_h = 2.0 ** (-8.0 * (np.arange(H) + 1) / H)
    slopes = np.tile(slopes_h, T)  # [Q]
    s_q = scale * inv_t  # [Q]

    q_eff = queries.reshape(Q, D) + bc.reshape(Q, D)  # [Q, D]
    qk0 = q_eff @ Wk  # [Q, D]
    # maug[c, q, d]: rows 0..C-1 = SC*s_q * (Wc_q^T @ Wk); row C = SC*s_q * qk0
    maug = np.empty((C + 1, Q, D), np.float64)
    for q in range(Q):
        Wc_q = Wc[q * D : (q + 1) * D, :]  # [D(e), C]
        maug[:C, q, :] = (Wc_q.T @ Wk) * (SC * s_q[q])
        maug[C, q, :] = qk0[q] * (SC * s_q[q])

    # sbias = SC*slope*s - MASK_NEG; the mask matmul adds back +MASK_NEG on
    # unmasked positions. All terms are power-of-two-scaled ints => exact f32.
    sbias = np.empty((128, S), np.float32)
    slope_col = np.tile(slopes, 128 // Q)  # [128], p -> slopes[p % 32]
    sbias[:] = SC * slope_col[:, None] * np.arange(S)[None, :] - MASK_NEG

    mstat = np.zeros((GRP, 128), np.float16)
    for r in range(GRP):
        mstat[r, 32 * r : 32 * (r + 1)] = MASK_NEG

    return dict(
        maug=maug.astype(np.float16),
        wvt=np.ascontiguousarray(Wv.T).astype(np.float32),
        sbias=sbias,
        mstat=mstat,
        id16=np.eye(128, dtype=np.float16),
        id32=np.eye(128, dtype=np.float32),
    )


def make_in_maps(keys, mask, context, consts, rows, n_cores):
    keys = np.asarray(keys, np.float32)
    mask_u8 = np.asarray(mask).astype(np.uint8)
    ctx = np.asarray(context, np.float32)
    in_maps = []
    for i in range(n_cores):
        sl = slice(i * rows, (i + 1) * rows)
        in_maps.append(
            dict(
                keys=np.ascontiguousarray(keys[sl]),
                mask=np.ascontiguousarray(mask_u8[sl]),
                ctx=np.ascontiguousarray(ctx[sl]),
                **consts,
            )
        )
    return in_maps


_cache = {}


def run(keys, mask, context, queries, Wk, Wv, log_temperature, Wc, bc,
        trace=False, **kw):
    consts = host_consts(queries, Wk, log_temperature, Wc, bc, Wv)
    if ROWS not in _cache:
        _cache[ROWS] = _build(ROWS)
    nc = _cache[ROWS]
    in_maps = make_in_maps(keys, mask, context, consts, ROWS, N_CORES)
    res = run_bass_kernel_spmd(nc, in_maps, core_ids=list(range(N_CORES)),
                               trace=trace, **kw)
    out = np.concatenate([res.results[i]["out"] for i in range(N_CORES)], axis=0)
    return out.reshape(B, T, H * D).astype(np.float32), res


def kernel(keys, mask, context, queries, Wk, Wv, log_temperature, Wc, bc):
    out, _ = run(keys, mask, context, queries, Wk, Wv, log_temperature, Wc, bc)
    return out



# revision 3
# speedup vs baseline: 1.2125x; 1.2125x over previous
"""AttentionPool TRN2 kernel.

Problem: B=2048, S=512, D=128, H=4, T=8 (Q = T*H = 32), C=64.
  k = keys @ Wk^T ; v = keys @ Wv^T
  q = q_flat + (ctx @ Wc^T + bc).reshape(B, Q, D)
  attn = (q @ k^T) * scale * inv_t[q] - slopes[q] * games_ago[s]
  out  = softmax_masked(attn) @ v            -> [B, T, H*D]

Structure (v2):
  - Host pre-casts keys to f16 and ships BOTH orientations:
      kt [rows, D, S]        (logits rhs: contract d)
      kp [rows, 128, 4, 128] (pass2 stationary tiles, s = c*128 + p)
    Same DMA bytes as one f32 copy, but no on-device cast and no PE
    key transposes.
  - All additive logit terms (ALiBi slope*s, -slope*(n-1) shift, mask)
    ride a single [6,*] matmul into the logits PSUM: rows 0-3 select
    MASK_NEG*mask[r,s], row 4 adds SC*slope_q*s, row 5 adds the
    per-(r,q) constant -(MASK_NEG + SC*slope_q*(n_r-1)) (host-computed
    from the mask). Softmax then needs no row-max: true logits <= ~2.
  - exp on scalar engine -> f16 weights + f32 row sums in one pass.
  - w^T via f16 PE transposes; pooled = (w @ keys) @ Wv^T in f16.

Sharding: pure data parallel over batch, 256 rows/core on 8 cores.
"""

import sys

if "/opt/trn_rl_repo" not in sys.path:
    sys.path.insert(0, "/opt/trn_rl_repo")

import numpy as np

import concourse.bacc as bacc
import concourse.bass as bass
import concourse.tile as tile
from concourse import mybir
from concourse.bass_utils import run_bass_kernel_spmd

B, S, D, H, T, C = 2048, 512, 128, 4, 8, 64
Q = T * H  # 32
N_CORES = 8
ROWS = B // N_CORES  # 256 rows per core
GRP = 4  # batch rows per group -> 4*32 = 128 partitions
BLK = 128  # rows per block (ctx/QKT staging)
SC = 64.0  # power-of-two prescale keeping f16 operands in normal range
MASK_NEG = 16384.0  # f16-exact; /SC = 256 pushes masked logits below -126

F32 = mybir.dt.float32
F16 = mybir.dt.float16

NCH = S // 128  # 4 s-chunks


def _emit(nc, tc, rows):
    """Emit the per-core program for `rows` batch rows (rows % GRP == 0)."""
    kt_d = nc.declare_dram_parameter("kt", [rows, D, S], F16, isOutput=False)
    kp_d = nc.declare_dram_parameter("kp", [rows, 128, NCH, D], F16, isOutput=False)
    ctx_d = nc.declare_dram_parameter("ctx", [rows, C], F32, isOutput=False)
    mrhs_d = nc.declare_dram_parameter("mrhs", [rows // GRP, 6, S], F16, isOutput=False)
    mlhs_d = nc.declare_dram_parameter("mlhs", [rows // GRP, 6, 128], F16, isOutput=False)
    maug_d = nc.declare_dram_parameter("maug", [C + 1, Q, D], F16, isOutput=False)
    wvt_d = nc.declare_dram_parameter("wvt", [D, D], F16, isOutput=False)
    id16_d = nc.declare_dram_parameter("id16", [128, 128], F16, isOutput=False)
    id32_d = nc.declare_dram_parameter("id32", [128, 128], F32, isOutput=False)
    out_d = nc.declare_dram_parameter("out", [rows, Q * D], F32, isOutput=True)

    kt_ap = kt_d.ap()
    kp_ap = kp_d.ap()
    ctx_ap = ctx_d.ap()
    out_ap = out_d.ap()

    n_blk = (rows + BLK - 1) // BLK

    import contextlib

    with contextlib.ExitStack() as ctx:
        singles = ctx.enter_context(tc.tile_pool(name="singles", bufs=1))
        kpool = ctx.enter_context(tc.tile_pool(name="kpool", bufs=4))
        mpool = ctx.enter_context(tc.tile_pool(name="mpool", bufs=6))
        blkpool = ctx.enter_context(tc.tile_pool(name="blkpool", bufs=2))
        qktpool = ctx.enter_context(tc.tile_pool(name="qktpool", bufs=2))
        work = ctx.enter_context(tc.tile_pool(name="work", bufs=3))
        small = ctx.enter_context(tc.tile_pool(name="small", bufs=4))
        ps = ctx.enter_context(tc.tile_pool(name="ps", bufs=1, space="PSUM"))

        # ---- constants (loaded once) ----
        maug_sb = singles.tile([C + 1, Q, D], F16)
        nc.sync.dma_start(out=maug_sb, in_=maug_d.ap())
        wvt_sb = singles.tile([D, D], F16)
        nc.sync.dma_start(out=wvt_sb, in_=wvt_d.ap())
        id16_sb = singles.tile([128, 128], F16)
        nc.sync.dma_start(out=id16_sb, in_=id16_d.ap())
        id32_sb = singles.tile([128, 128], F32)
        nc.sync.dma_start(out=id32_sb, in_=id32_d.ap())

        # ---- prologue: conditioned queries qk'^T for every block ----
        qkt_blocks = []
        for blk in range(n_blk):
            r0 = blk * BLK
            bn = min(BLK, rows - r0)
            assert bn % GRP == 0

            ctx_sb = blkpool.tile([BLK, C], F32, tag="ctx")
            nc.sync.dma_start(out=ctx_sb[:bn], in_=ctx_ap[r0 : r0 + bn])
            ctxt_ps = ps.tile([C, BLK], F32, tag="smallf32", bufs=1)
            nc.tensor.transpose(ctxt_ps[:, :bn], ctx_sb[:bn], id32_sb[:bn, :bn])
            ctxt_sb = blkpool.tile([C + 1, BLK], F16, tag="ctxt")
            nc.vector.tensor_copy(out=ctxt_sb[:C, :bn], in_=ctxt_ps[:, :bn])
            nc.vector.memset(ctxt_sb[C : C + 1, :bn], 1.0)

            # qk'^T for the block: [D, bn, Q] f16 (prescaled by SC*scale*inv_t)
            qkt_sb = qktpool.tile([D, BLK, Q], F16, tag="qkt")
            for q in range(Q):
                qkt_ps = ps.tile([D, BLK], F32, tag="smallf32", bufs=1)
                nc.tensor.matmul(
                    qkt_ps[:, :bn], maug_sb[:, q, :], ctxt_sb[:, :bn],
                    start=True, stop=True,
                )
                nc.vector.tensor_copy(out=qkt_sb[:, :bn, q], in_=qkt_ps[:, :bn])
            qkt_blocks.append(qkt_sb)

        n_grp_total = rows // GRP
        PF = 3  # software prefetch distance (groups)
        staged = {}

        def _load_group(g):
            if g >= n_grp_total or g in staged:
                return
            g0 = g * GRP
            ktg = kpool.tile([128, GRP, S], F16, tag="ktg", name=f"ktg_{g}")
            for r in range(GRP):
                nc.sync.dma_start(out=ktg[:, r], in_=kt_ap[g0 + r])
            kpg = kpool.tile([128, GRP, NCH, D], F16, tag="kpg", name=f"kpg_{g}")
            for r in range(GRP):
                nc.sync.dma_start(out=kpg[:, r], in_=kp_ap[g0 + r])
            mr = mpool.tile([6, S], F16, tag="mrhs", name=f"mr_{g}")
            nc.sync.dma_start(out=mr, in_=mrhs_d.ap()[g])
            ml = mpool.tile([6, 128], F16, tag="mlhs", name=f"ml_{g}")
            nc.sync.dma_start(out=ml, in_=mlhs_d.ap()[g])
            staged[g] = (ktg, kpg, mr, ml)

        for g in range(PF):
            _load_group(g)

        for g in range(n_grp_total):
            g0 = g * GRP  # absolute row of this group
            qkt_sb = qkt_blocks[g0 // BLK]
            gl = g0 % BLK  # row offset inside the block
            _load_group(g + PF)
            ktg, kpg, mr, ml = staged.pop(g)

            # ---- logits psum: qk' . k^T  (+ bias/mask matmul) ----
            lg_ps = ps.tile([128, S], F32, tag="logits", bufs=2)
            for r in range(GRP):
                nc.tensor.matmul(
                    lg_ps[32 * r : 32 * (r + 1), :],
                    qkt_sb[:, gl + r, :],
                    ktg[:, r, :],
                    start=True, stop=False,
                    tile_position=(0, 32 * r),
                    skip_group_check=True,
                )
            nc.tensor.matmul(
                lg_ps, ml, mr,
                start=False, stop=True,
                skip_group_check=True,
            )

            # ---- softmax (no row max needed: true logits <= ~2) ----
            e16 = work.tile([128, S], F16, tag="e16")
            sum_sb = small.tile([128, 1], F32, tag="sum")
            nc.scalar.activation(
                out=e16, in_=lg_ps, func=mybir.ActivationFunctionType.Exp,
                scale=1.0 / SC, accum_out=sum_sb,
            )
            rs_sb = small.tile([128, 1], F32, tag="rs")
            nc.vector.reciprocal(rs_sb, sum_sb)

            # ---- w^T: [s_in_chunk, c, rq] f16 via PE transposes ----
            wt_ps = ps.tile([128, NCH, 128], F16, tag="wtps", bufs=2)
            for c in range(NCH):
                nc.tensor.transpose(
                    wt_ps[:, c, :], e16[:, c * 128 : (c + 1) * 128], id16_sb
                )
            wt16 = work.tile([128, NCH, 128], F16, tag="wt")
            nc.scalar.copy(out=wt16, in_=wt_ps)

            # ---- pass 2: pk[d, rq] = sum_s keys[s,d] * w[s,rq] ----
            pk_ps = ps.tile([128, 128], F32, tag="pk", bufs=2)
            for r in range(GRP):
                for c in range(NCH):
                    nc.tensor.matmul(
                        pk_ps[:, 32 * r : 32 * (r + 1)],
                        kpg[:, r, c, :],
                        wt16[:, c, 32 * r : 32 * (r + 1)],
                        start=(c == 0), stop=(c == NCH - 1),
                        skip_group_check=True,
                    )
            pkt16 = work.tile([128, 128], F16, tag="pkt")
            nc.vector.tensor_copy(out=pkt16, in_=pk_ps)

            # ---- pooled[rq, e] = pk^T @ Wv^T, scaled by 1/rowsum ----
            po_ps = ps.tile([128, 128], F32, tag="po", bufs=1)
            nc.tensor.matmul(po_ps, pkt16, wvt_sb, start=True, stop=True)

            o_sb = work.tile([128, 128], F32, tag="o")
            nc.vector.tensor_scalar(
                out=o_sb, in0=po_ps, scalar1=rs_sb, scalar2=None,
                op0=mybir.AluOpType.mult,
            )
            nc.scalar.dma_start(
                out=out_ap[g0 : g0 + GRP].rearrange("r (q e) -> (r q) e", e=D),
                in_=o_sb,
            )


def _build(rows):
    nc = bacc.Bacc(trn_type="TRN2", target_bir_lowering=False, debug=False)
    with tile.TileContext(nc) as tc:
        _emit(nc, tc, rows)
    nc.compile()
    return nc


def host_consts(queries, Wk, log_temperature, Wc, bc, Wv):
    """Fold projections/scales into small host-side constants."""
    queries = np.asarray(queries, np.float64)
    Wk = np.asarray(Wk, np.float64)
    Wc = np.asarray(Wc, np.float64)
    bc = np.asarray(bc, np.float64)
    Wv = np.asarray(Wv, np.float64)
    lt = np.asarray(log_temperature, np.float64)

    scale = D ** -0.5
    inv_t = np.repeat(np.exp(-lt), H)  # [Q]
    s_q = scale * inv_t  # [Q]

    q_eff = queries.reshape(Q, D) + bc.reshape(Q, D)  # [Q, D]
    qk0 = q_eff @ Wk  # [Q, D]
    # maug[c, q, d]: rows 0..C-1 = SC*s_q * (Wc_q^T @ Wk); row C = SC*s_q * qk0
    maug = np.empty((C + 1, Q, D), np.float64)
    for q in range(Q):
        Wc_q = Wc[q * D : (q + 1) * D, :]  # [D(e), C]
        maug[:C, q, :] = (Wc_q.T @ Wk) * (SC * s_q[q])
        maug[C, q, :] = qk0[q] * (SC * s_q[q])

    return dict(
        maug=maug.astype(np.float16),
        wvt=np.ascontiguousarray(Wv.T).astype(np.float16),
        id16=np.eye(128, dtype=np.float16),
        id32=np.eye(128, dtype=np.float32),
    )


def _slopes_q():
    slopes_h = 2.0 ** (-8.0 * (np.arange(H) + 1) / H)
    return np.tile(slopes_h, T)  # [Q]


def make_in_maps(keys, mask, context, consts, rows, n_cores):
    keys16 = np.asarray(keys, np.float32).astype(np.float16)  # [B, S, D]
    kt = np.ascontiguousarray(keys16.transpose(0, 2, 1))  # [B, D, S]
    kp = np.ascontiguousarray(
        keys16.reshape(B, NCH, 128, D).transpose(0, 2, 1, 3)
    )  # [B, 128(p), NCH(c), D], s = c*128 + p
    mask_b = np.asarray(mask).astype(bool)
    ctx = np.asarray(context, np.float32)

    n_real = mask_b.sum(axis=1).astype(np.float64)  # [B]
    slopes = _slopes_q()  # [Q]
    n_grp = rows // GRP

    # mrhs rows 0-3: mask[r] as f16; row 4: s values; row 5: ones
    # mlhs rows 0-3: MASK_NEG on the r-th 32-col block
    #      row 4: SC*slope_q ; row 5: -(MASK_NEG + SC*slope_q*(n_r - 1))
    svals = np.arange(S, dtype=np.float64)

    in_maps = []
    for i in range(n_cores):
        sl = slice(i * rows, (i + 1) * rows)
        mk = mask_b[sl]  # [rows, S]
        nr = n_real[sl]  # [rows]
        mrhs = np.zeros((n_grp, 6, S), np.float16)
        mlhs = np.zeros((n_grp, 6, 128), np.float16)
        for g in range(n_grp):
            g0 = g * GRP
            mrhs[g, :GRP] = mk[g0 : g0 + GRP].astype(np.float16)
            mrhs[g, 4] = svals.astype(np.float16)
            mrhs[g, 5] = 1.0
            for r in range(GRP):
                mlhs[g, r, 32 * r : 32 * (r + 1)] = MASK_NEG
                mlhs[g, 5, 32 * r : 32 * (r + 1)] = -(
                    MASK_NEG + SC * slopes * (nr[g0 + r] - 1.0)
                ).astype(np.float16)
            mlhs[g, 4] = np.tile(SC * slopes, 128 // Q).astype(np.float16)
        in_maps.append(
            dict(
                kt=np.ascontiguousarray(kt[sl]),
                kp=np.ascontiguousarray(kp[sl]),
                ctx=np.ascontiguousarray(ctx[sl]),
                mrhs=mrhs,
                mlhs=mlhs,
                **consts,
            )
        )
    return in_maps


_cache = {}


def run(keys, mask, context, queries, Wk, Wv, log_temperature, Wc, bc,
        trace=False, **kw):
    consts = host_consts(queries, Wk, log_temperature, Wc, bc, Wv)
    if ROWS not in _cache:
        _cache[ROWS] = _build(ROWS)
    nc = _cache[ROWS]
    in_maps = make_in_maps(keys, mask, context, consts, ROWS, N_CORES)
    res = run_bass_kernel_spmd(nc, in_maps, core_ids=list(range(N_CORES)),
                               trace=trace, **kw)
    out = np.concatenate([res.results[i]["out"] for i in range(N_CORES)], axis=0)
    return out.reshape(B, T, H * D).astype(np.float32), res


def kernel(keys, mask, context, queries, Wk, Wv, log_temperature, Wc, bc):
    out, _ = run(keys, mask, context, queries, Wk, Wv, log_temperature, Wc, bc)
    return out


# revision 8
# speedup vs baseline: 1.8170x; 1.4985x over previous
"""AttentionPool TRN2 kernel.

Problem: B=2048, S=512, D=128, H=4, T=8 (Q = T*H = 32), C=64.
  k = keys @ Wk^T ; v = keys @ Wv^T
  q = q_flat + (ctx @ Wc^T + bc).reshape(B, Q, D)
  attn = (q @ k^T) * scale * inv_t[q] - slopes[q] * games_ago[s]
  out  = softmax_masked(attn) @ v            -> [B, T, H*D]

Structure (v2):
  - Host pre-casts keys to f16 and ships BOTH orientations:
      kt [rows, D, S]        (logits rhs: contract d)
      kp [rows, 128, 4, 128] (pass2 stationary tiles, s = c*128 + p)
    Same DMA bytes as one f32 copy, but no on-device cast and no PE
    key transposes.
  - All additive logit terms (ALiBi slope*s, -slope*(n-1) shift, mask)
    ride a single [6,*] matmul into the logits PSUM: rows 0-3 select
    MASK_NEG*mask[r,s], row 4 adds SC*slope_q*s, row 5 adds the
    per-(r,q) constant -(MASK_NEG + SC*slope_q*(n_r-1)) (host-computed
    from the mask). Softmax then needs no row-max: true logits <= ~2.
  - exp on scalar engine -> f16 weights + f32 row sums in one pass.
  - w^T via f16 PE transposes; pooled = (w @ keys) @ Wv^T in f16.

Sharding: pure data parallel over batch, 256 rows/core on 8 cores.
"""

import sys

if "/opt/trn_rl_repo" not in sys.path:
    sys.path.insert(0, "/opt/trn_rl_repo")

import numpy as np

import concourse.bacc as bacc
import concourse.bass as bass
import concourse.tile as tile
from concourse import mybir
from concourse.bass_utils import run_bass_kernel_spmd

B, S, D, H, T, C = 2048, 512, 128, 4, 8, 64
Q = T * H  # 32
N_CORES = 8
ROWS = B // N_CORES  # 256 rows per core
GRP = 4  # batch rows per group -> 4*32 = 128 partitions
BLK = 128  # rows per block (ctx/QKT staging)
SC = 64.0  # power-of-two prescale keeping f16 operands in normal range
MASK_NEG = 16384.0  # f16-exact; /SC = 256 pushes masked logits below -126

F32 = mybir.dt.float32
F16 = mybir.dt.float16

NCH = S // 128  # 4 s-chunks


def _emit(nc, tc, rows):
    """Emit the per-core program for `rows` batch rows (rows % GRP == 0).

    kk[row] packs both key orientations: [:, 0, :] = keys^T row ([d, s]),
    [:, 1, :] = pass2 tiles ([p, (c d)], s = c*128 + p). m6 packs the
    bias/mask matmul rhs (cols 0..S-1) and lhsT (cols S..S+127).
    """
    kk_d = nc.declare_dram_parameter("kk", [rows, 128, 2, S], F16, isOutput=False)
    ctx_d = nc.declare_dram_parameter("ctx", [rows, C], F32, isOutput=False)
    m6_d = nc.declare_dram_parameter("m6", [rows // GRP, 6, S + 128], F16, isOutput=False)
    maug_d = nc.declare_dram_parameter("maug", [C + 1, Q, D], F16, isOutput=False)
    wvt_d = nc.declare_dram_parameter("wvt", [D, D], F16, isOutput=False)
    id16_d = nc.declare_dram_parameter("id16", [128, 128], F16, isOutput=False)
    id32_d = nc.declare_dram_parameter("id32", [128, 128], F32, isOutput=False)
    out_d = nc.declare_dram_parameter("out", [rows, Q * D], F32, isOutput=True)

    kk_ap = kk_d.ap()
    ctx_ap = ctx_d.ap()
    out_ap = out_d.ap()

    n_blk = (rows + BLK - 1) // BLK

    import contextlib

    with contextlib.ExitStack() as ctx:
        singles = ctx.enter_context(tc.tile_pool(name="singles", bufs=1))
        kpool = ctx.enter_context(tc.tile_pool(name="kpool", bufs=4))
        mpool = ctx.enter_context(tc.tile_pool(name="mpool", bufs=6))
        blkpool = ctx.enter_context(tc.tile_pool(name="blkpool", bufs=2))
        qktpool = ctx.enter_context(tc.tile_pool(name="qktpool", bufs=2))
        work = ctx.enter_context(tc.tile_pool(name="work", bufs=3))
        small = ctx.enter_context(tc.tile_pool(name="small", bufs=4))
        ps = ctx.enter_context(tc.tile_pool(name="ps", bufs=1, space="PSUM"))

        # ---- constants (loaded once) ----
        maug_sb = singles.tile([C + 1, Q, D], F16)
        nc.sync.dma_start(out=maug_sb, in_=maug_d.ap())
        wvt_sb = singles.tile([D, D], F16)
        nc.sync.dma_start(out=wvt_sb, in_=wvt_d.ap())
        id16_sb = singles.tile([128, 128], F16)
        nc.sync.dma_start(out=id16_sb, in_=id16_d.ap())
        id32_sb = singles.tile([128, 128], F32)
        nc.sync.dma_start(out=id32_sb, in_=id32_d.ap())

        # ---- prologue: conditioned queries qk'^T for every block ----
        qkt_blocks = []
        for blk in range(n_blk):
            r0 = blk * BLK
            bn = min(BLK, rows - r0)
            assert bn % GRP == 0

            ctx_sb = blkpool.tile([BLK, C], F32, tag="ctx")
            nc.sync.dma_start(out=ctx_sb[:bn], in_=ctx_ap[r0 : r0 + bn])
            ctxt_ps = ps.tile([C, BLK], F32, tag="smallf32", bufs=1)
            nc.tensor.transpose(ctxt_ps[:, :bn], ctx_sb[:bn], id32_sb[:bn, :bn])
            ctxt_sb = blkpool.tile([C + 1, BLK], F16, tag="ctxt")
            nc.vector.tensor_copy(out=ctxt_sb[:C, :bn], in_=ctxt_ps[:, :bn])
            nc.vector.memset(ctxt_sb[C : C + 1, :bn], 1.0)

            # qk'^T for the block: [D, bn, Q] f16 (prescaled by SC*scale*inv_t)
            qkt_sb = qktpool.tile([D, BLK, Q], F16, tag="qkt")
            for q in range(Q):
                qkt_ps = ps.tile([D, BLK], F32, tag="smallf32", bufs=1)
                nc.tensor.matmul(
                    qkt_ps[:, :bn], maug_sb[:, q, :], ctxt_sb[:, :bn],
                    start=True, stop=True,
                )
                nc.vector.tensor_copy(out=qkt_sb[:, :bn, q], in_=qkt_ps[:, :bn])
            qkt_blocks.append(qkt_sb)

        n_grp_total = rows // GRP
        PF = 3  # software prefetch distance (groups)
        staged = {}

        def _load_group(g):
            if g >= n_grp_total or g in staged:
                return
            g0 = g * GRP
            kkg = kpool.tile([128, GRP, 2, S], F16, tag="kk", name=f"kk_{g}")
            nc.sync.dma_start(
                out=kkg,
                in_=kk_ap[g0 : g0 + GRP].rearrange("r p h s -> p r h s"),
            )
            m6 = mpool.tile([6, S + 128], F16, tag="m6", name=f"m6_{g}")
            nc.gpsimd.dma_start(out=m6, in_=m6_d.ap()[g])
            staged[g] = (kkg, m6)

        for g in range(PF):
            _load_group(g)

        for g in range(n_grp_total):
            g0 = g * GRP  # absolute row of this group
            qkt_sb = qkt_blocks[g0 // BLK]
            gl = g0 % BLK  # row offset inside the block
            _load_group(g + PF)
            kkg, m6 = staged.pop(g)

            # ---- logits psum: qk' . k^T  (+ bias/mask matmul) ----
            lg_ps = ps.tile([128, S], F32, tag="logits", bufs=2)
            for r in range(GRP):
                nc.tensor.matmul(
                    lg_ps[32 * r : 32 * (r + 1), :],
                    qkt_sb[:, gl + r, :],
                    kkg[:, r, 0, :],
                    start=True, stop=False,
                    tile_position=(0, 32 * r),
                    skip_group_check=True,
                )
            nc.tensor.matmul(
                lg_ps, m6[:, S : S + 128], m6[:, :S],
                start=False, stop=True,
                skip_group_check=True,
            )

            # ---- softmax (no row max needed: true logits <= ~2) ----
            e16 = work.tile([128, S], F16, tag="e16")
            sum_sb = small.tile([128, 1], F32, tag="sum")
            nc.scalar.activation(
                out=e16, in_=lg_ps, func=mybir.ActivationFunctionType.Exp,
                scale=1.0 / SC, accum_out=sum_sb,
            )
            rs_sb = small.tile([128, 1], F32, tag="rs")
            nc.vector.reciprocal(rs_sb, sum_sb)

            # ---- w^T: [s_in_chunk, c, rq] f16 via PE transposes ----
            wt_ps = ps.tile([128, NCH, 128], F16, tag="wtps", bufs=2)
            for c in range(NCH):
                nc.tensor.transpose(
                    wt_ps[:, c, :], e16[:, c * 128 : (c + 1) * 128], id16_sb
                )
            wt16 = work.tile([128, NCH, 128], F16, tag="wt")
            nc.scalar.copy(out=wt16, in_=wt_ps)

            # ---- pass 2: pk[d, rq] = sum_s keys[s,d] * w[s,rq] ----
            pk_ps = ps.tile([128, 128], F32, tag="pk", bufs=2)
            for r in range(GRP):
                for c in range(NCH):
                    nc.tensor.matmul(
                        pk_ps[:, 32 * r : 32 * (r + 1)],
                        kkg[:, r, 1, c * 128 : (c + 1) * 128],
                        wt16[:, c, 32 * r : 32 * (r + 1)],
                        start=(c == 0), stop=(c == NCH - 1),
                        skip_group_check=True,
                    )
            pkt16 = work.tile([128, 128], F16, tag="pkt")
            nc.vector.tensor_copy(out=pkt16, in_=pk_ps)

            # ---- pooled[rq, e] = pk^T @ Wv^T, scaled by 1/rowsum ----
            po_ps = ps.tile([128, 128], F32, tag="po", bufs=1)
            nc.tensor.matmul(po_ps, pkt16, wvt_sb, start=True, stop=True)

            o_sb = work.tile([128, 128], F32, tag="o")
            nc.vector.tensor_scalar(
                out=o_sb, in0=po_ps, scalar1=rs_sb, scalar2=None,
                op0=mybir.AluOpType.mult,
            )
            nc.scalar.dma_start(
                out=out_ap[g0 : g0 + GRP].rearrange("r (q e) -> (r q) e", e=D),
                in_=o_sb,
            )


def _build(rows):
    nc = bacc.Bacc(trn_type="TRN2", target_bir_lowering=False, debug=False)
    with tile.TileContext(nc) as tc:
        _emit(nc, tc, rows)
    nc.compile()
    return nc


def host_consts(queries, Wk, log_temperature, Wc, bc, Wv):
    """Fold projections/scales into small host-side constants."""
    queries = np.asarray(queries, np.float64)
    Wk = np.asarray(Wk, np.float64)
    Wc = np.asarray(Wc, np.float64)
    bc = np.asarray(bc, np.float64)
    Wv = np.asarray(Wv, np.float64)
    lt = np.asarray(log_temperature, np.float64)

    scale = D ** -0.5
    inv_t = np.repeat(np.exp(-lt), H)  # [Q]
    s_q = scale * inv_t  # [Q]

    q_eff = queries.reshape(Q, D) + bc.reshape(Q, D)  # [Q, D]
    qk0 = q_eff @ Wk  # [Q, D]
    # maug[c, q, d]: rows 0..C-1 = SC*s_q * (Wc_q^T @ Wk); row C = SC*s_q * qk0
    maug = np.empty((C + 1, Q, D), np.float64)
    for q in range(Q):
        Wc_q = Wc[q * D : (q + 1) * D, :]  # [D(e), C]
        maug[:C, q, :] = (Wc_q.T @ Wk) * (SC * s_q[q])
        maug[C, q, :] = qk0[q] * (SC * s_q[q])

    return dict(
        maug=maug.astype(np.float16),
        wvt=np.ascontiguousarray(Wv.T).astype(np.float16),
        id16=np.eye(128, dtype=np.float16),
        id32=np.eye(128, dtype=np.float32),
    )


def _slopes_q():
    slopes_h = 2.0 ** (-8.0 * (np.arange(H) + 1) / H)
    return np.tile(slopes_h, T)  # [Q]


def make_in_maps(keys, mask, context, consts, rows, n_cores):
    keys16 = np.asarray(keys, np.float32).astype(np.float16)  # [B, S, D]
    # kk[:, :, 0, :] = keys^T ([d, s]); kk[:, :, 1, :] = pass2 tiles
    # ([p, (c d)], s = c*128 + p). One DMA per group covers both.
    kk = np.empty((B, 128, 2, S), np.float16)
    kk[:, :, 0, :] = keys16.transpose(0, 2, 1)
    kk[:, :, 1, :] = (
        keys16.reshape(B, NCH, 128, D).transpose(0, 2, 1, 3).reshape(B, 128, S)
    )
    mask_b = np.asarray(mask).astype(bool)
    ctx = np.asarray(context, np.float32)

    n_real = mask_b.sum(axis=1).astype(np.float64)  # [B]
    slopes = _slopes_q()  # [Q]
    n_grp = rows // GRP

    # m6[:, :, :S] (rhs)  rows 0-3: mask[r] f16; row 4: s values; row 5: ones
    # m6[:, :, S:] (lhsT) rows 0-3: MASK_NEG on the r-th 32-col block;
    #      row 4: SC*slope_q ; row 5: -(MASK_NEG + SC*slope_q*(n_r - 1))
    svals = np.arange(S, dtype=np.float64)

    in_maps = []
    for i in range(n_cores):
        sl = slice(i * rows, (i + 1) * rows)
        mk = mask_b[sl]  # [rows, S]
        nr = n_real[sl]  # [rows]
        m6 = np.zeros((n_grp, 6, S + 128), np.float16)
        for g in range(n_grp):
            g0 = g * GRP
            m6[g, :GRP, :S] = mk[g0 : g0 + GRP].astype(np.float16)
            m6[g, 4, :S] = svals.astype(np.float16)
            m6[g, 5, :S] = 1.0
            for r in range(GRP):
                m6[g, r, S + 32 * r : S + 32 * (r + 1)] = MASK_NEG
                m6[g, 5, S + 32 * r : S + 32 * (r + 1)] = -(
                    MASK_NEG + SC * slopes * (nr[g0 + r] - 1.0)
                ).astype(np.float16)
            m6[g, 4, S:] = np.tile(SC * slopes, 128 // Q).astype(np.float16)
        in_maps.append(
            dict(
                kk=np.ascontiguousarray(kk[sl]),
                ctx=np.ascontiguousarray(ctx[sl]),
                m6=m6,
                **consts,
            )
        )
    return in_maps


_cache = {}


def run(keys, mask, context, queries, Wk, Wv, log_temperature, Wc, bc,
        trace=False, **kw):
    consts = host_consts(queries, Wk, log_temperature, Wc, bc, Wv)
    if ROWS not in _cache:
        _cache[ROWS] = _build(ROWS)
    nc = _cache[ROWS]
    in_maps = make_in_maps(keys, mask, context, consts, ROWS, N_CORES)
    res = run_bass_kernel_spmd(nc, in_maps, core_ids=list(range(N_CORES)),
                               trace=trace, **kw)
    out = np.concatenate([res.results[i]["out"] for i in range(N_CORES)], axis=0)
    return out.reshape(B, T, H * D).astype(np.float32), res


def kernel(keys, mask, context, queries, Wk, Wv, log_temperature, Wc, bc):
    out, _ = run(keys, mask, context, queries, Wk, Wv, log_temperature, Wc, bc)
    return out


# revision 18
# speedup vs baseline: 2.3119x; 1.2723x over previous
"""AttentionPool TRN2 kernel.

Problem: B=2048, S=512, D=128, H=4, T=8 (Q = T*H = 32), C=64.
  k = keys @ Wk^T ; v = keys @ Wv^T
  q = q_flat + (ctx @ Wc^T + bc).reshape(B, Q, D)
  attn = (q @ k^T) * scale * inv_t[q] - slopes[q] * games_ago[s]
  out  = softmax_masked(attn) @ v            -> [B, T, H*D]

Structure (v2):
  - Host pre-casts keys to f16 and ships BOTH orientations:
      kt [rows, D, S]        (logits rhs: contract d)
      kp [rows, 128, 4, 128] (pass2 stationary tiles, s = c*128 + p)
    Same DMA bytes as one f32 copy, but no on-device cast and no PE
    key transposes.
  - All additive logit terms (ALiBi slope*s, -slope*(n-1) shift, mask)
    ride a single [6,*] matmul into the logits PSUM: rows 0-3 select
    MASK_NEG*mask[r,s], row 4 adds SC*slope_q*s, row 5 adds the
    per-(r,q) constant -(MASK_NEG + SC*slope_q*(n_r-1)) (host-computed
    from the mask). Softmax then needs no row-max: true logits <= ~2.
  - exp on scalar engine -> f16 weights + f32 row sums in one pass.
  - w^T via f16 PE transposes; pooled = (w @ keys) @ Wv^T in f16.

Sharding: pure data parallel over batch, 256 rows/core on 8 cores.
"""

import sys

if "/opt/trn_rl_repo" not in sys.path:
    sys.path.insert(0, "/opt/trn_rl_repo")

import numpy as np

import concourse.bacc as bacc
import concourse.bass as bass
import concourse.tile as tile
from concourse import mybir
from concourse.bass_utils import run_bass_kernel_spmd

B, S, D, H, T, C = 2048, 512, 128, 4, 8, 64
Q = T * H  # 32
N_CORES = 8
ROWS = B // N_CORES  # 256 rows per core
GRP = 4  # batch rows per group -> 4*32 = 128 partitions
BLK = 128  # rows per block (ctx/QKT staging)
SC = 64.0  # power-of-two prescale keeping f16 operands in normal range
MASK_NEG = 16384.0  # f16-exact; /SC = 256 pushes masked logits below -126

F32 = mybir.dt.float32
F16 = mybir.dt.float16

NCH = S // 128  # 4 s-chunks


def _emit(nc, tc, rows, cc):
    """Emit the per-core program for `rows` batch rows (rows % GRP == 0).

    kk[row] packs both key orientations: [:, 0, :] = keys^T row ([d, s]),
    [:, 1, :] = pass2 tiles ([p, (c d)], s = c*128 + p). m6 packs the
    bias/mask matmul rhs (cols 0..S-1) and lhsT (cols S..S+127).

    cc[g] in 1..4 is the number of 128-wide s-chunks group g touches.
    Rows are host-sorted by length so every skipped chunk is fully
    masked; its softmax terms are exact zeros, so the result is
    identical to the full computation.
    """
    kk_d = nc.declare_dram_parameter(
        "kk", [rows // GRP, 128, GRP, 2, S], F16, isOutput=False
    )
    ctx_d = nc.declare_dram_parameter("ctx", [rows, C], F32, isOutput=False)
    m6_d = nc.declare_dram_parameter("m6", [rows // GRP, 6, S + 128], F16, isOutput=False)
    maug_d = nc.declare_dram_parameter("maug", [C + 1, Q, D], F16, isOutput=False)
    wvt_d = nc.declare_dram_parameter("wvt", [D, D], F16, isOutput=False)
    id16_d = nc.declare_dram_parameter("id16", [128, 128], F16, isOutput=False)
    id32_d = nc.declare_dram_parameter("id32", [128, 128], F32, isOutput=False)
    out_d = nc.declare_dram_parameter("out", [rows, Q * D], F32, isOutput=True)

    kk_ap = kk_d.ap()
    ctx_ap = ctx_d.ap()
    out_ap = out_d.ap()

    n_blk = (rows + BLK - 1) // BLK

    import contextlib

    with contextlib.ExitStack() as ctx:
        singles = ctx.enter_context(tc.tile_pool(name="singles", bufs=1))
        kpool = ctx.enter_context(tc.tile_pool(name="kpool", bufs=4))
        mpool = ctx.enter_context(tc.tile_pool(name="mpool", bufs=6))
        blkpool = ctx.enter_context(tc.tile_pool(name="blkpool", bufs=2))
        qktpool = ctx.enter_context(tc.tile_pool(name="qktpool", bufs=2))
        work = ctx.enter_context(tc.tile_pool(name="work", bufs=3))
        small = ctx.enter_context(tc.tile_pool(name="small", bufs=4))
        ps = ctx.enter_context(tc.tile_pool(name="ps", bufs=1, space="PSUM"))

        # ---- constants (loaded once) ----
        maug_sb = singles.tile([C + 1, Q, D], F16)
        nc.sync.dma_start(out=maug_sb, in_=maug_d.ap())
        wvt_sb = singles.tile([D, D], F16)
        nc.sync.dma_start(out=wvt_sb, in_=wvt_d.ap())
        id16_sb = singles.tile([128, 128], F16)
        nc.sync.dma_start(out=id16_sb, in_=id16_d.ap())
        id32_sb = singles.tile([128, 128], F32)
        nc.sync.dma_start(out=id32_sb, in_=id32_d.ap())

        # ---- prologue: conditioned queries qk'^T for every block ----
        qkt_blocks = []
        for blk in range(n_blk):
            r0 = blk * BLK
            bn = min(BLK, rows - r0)
            assert bn % GRP == 0

            ctx_sb = blkpool.tile([BLK, C], F32, tag="ctx")
            nc.sync.dma_start(out=ctx_sb[:bn], in_=ctx_ap[r0 : r0 + bn])
            ctxt_ps = ps.tile([C, BLK], F32, tag="smallf32", bufs=1)
            nc.tensor.transpose(ctxt_ps[:, :bn], ctx_sb[:bn], id32_sb[:bn, :bn])
            ctxt_sb = blkpool.tile([C + 1, BLK], F16, tag="ctxt")
            nc.vector.tensor_copy(out=ctxt_sb[:C, :bn], in_=ctxt_ps[:, :bn])
            nc.vector.memset(ctxt_sb[C : C + 1, :bn], 1.0)

            # qk'^T for the block: [D, bn, Q] f16 (prescaled by SC*scale*inv_t)
            qkt_sb = qktpool.tile([D, BLK, Q], F16, tag="qkt")
            for q in range(Q):
                qkt_ps = ps.tile([D, BLK], F32, tag="smallf32", bufs=1)
                nc.tensor.matmul(
                    qkt_ps[:, :bn], maug_sb[:, q, :], ctxt_sb[:, :bn],
                    start=True, stop=True,
                )
                nc.vector.tensor_copy(out=qkt_sb[:, :bn, q], in_=qkt_ps[:, :bn])
            qkt_blocks.append(qkt_sb)

        n_grp_total = rows // GRP
        PF = 3  # software prefetch distance (groups)
        staged = {}

        def _load_group(g):
            if g >= n_grp_total or g in staged:
                return
            sl = cc[g] * 128
            kkg = kpool.tile([128, GRP, 2, S], F16, tag="kk", name=f"kk_{g}")
            nc.sync.dma_start(
                out=kkg[:, :, :, :sl],
                in_=kk_ap[g, :, :, :, :sl],
            )
            m6 = mpool.tile([6, S + 128], F16, tag="m6", name=f"m6_{g}")
            nc.gpsimd.dma_start(out=m6, in_=m6_d.ap()[g])
            staged[g] = (kkg, m6)

        for g in range(PF):
            _load_group(g)

        for g in range(n_grp_total):
            g0 = g * GRP  # absolute row of this group
            qkt_sb = qkt_blocks[g0 // BLK]
            gl = g0 % BLK  # row offset inside the block
            _load_group(g + PF)
            kkg, m6 = staged.pop(g)
            ncg = cc[g]
            sl = ncg * 128

            # ---- logits psum: qk' . k^T  (+ bias/mask matmul) ----
            lg_ps = ps.tile([128, S], F32, tag="logits", bufs=2)
            for r in range(GRP):
                nc.tensor.matmul(
                    lg_ps[32 * r : 32 * (r + 1), :sl],
                    qkt_sb[:, gl + r, :],
                    kkg[:, r, 0, :sl],
                    start=True, stop=False,
                    tile_position=(0, 32 * r),
                    skip_group_check=True,
                )
            nc.tensor.matmul(
                lg_ps[:, :sl], m6[:, S : S + 128], m6[:, :sl],
                start=False, stop=True,
                skip_group_check=True,
            )

            # ---- softmax (no row max needed: true logits <= ~2) ----
            e16 = work.tile([128, S], F16, tag="e16")
            sum_sb = small.tile([128, 1], F32, tag="sum")
            nc.scalar.activation(
                out=e16[:, :sl], in_=lg_ps[:, :sl],
                func=mybir.ActivationFunctionType.Exp,
                scale=1.0 / SC, accum_out=sum_sb,
            )
            rs_sb = small.tile([128, 1], F32, tag="rs")
            nc.vector.reciprocal(rs_sb, sum_sb)

            # ---- w^T: [s_in_chunk, c, rq] f16 via PE transposes ----
            wt_ps = ps.tile([128, NCH, 128], F16, tag="wtps", bufs=2)
            for c in range(ncg):
                nc.tensor.transpose(
                    wt_ps[:, c, :], e16[:, c * 128 : (c + 1) * 128], id16_sb
                )
            wt16 = work.tile([128, NCH, 128], F16, tag="wt")
            nc.scalar.copy(out=wt16[:, :ncg], in_=wt_ps[:, :ncg])

            # ---- pass 2: pk[d, rq] = sum_s keys[s,d] * w[s,rq] ----
            pk_ps = ps.tile([128, 128], F32, tag="pk", bufs=2)
            for r in range(GRP):
                for c in range(ncg):
                    nc.tensor.matmul(
                        pk_ps[:, 32 * r : 32 * (r + 1)],
                        kkg[:, r, 1, c * 128 : (c + 1) * 128],
                        wt16[:, c, 32 * r : 32 * (r + 1)],
                        start=(c == 0), stop=(c == ncg - 1),
                        skip_group_check=True,
                    )
            pkt16 = work.tile([128, 128], F16, tag="pkt")
            nc.vector.tensor_copy(out=pkt16, in_=pk_ps)

            # ---- pooled[rq, e] = pk^T @ Wv^T, scaled by 1/rowsum ----
            po_ps = ps.tile([128, 128], F32, tag="po", bufs=1)
            nc.tensor.matmul(po_ps, pkt16, wvt_sb, start=True, stop=True)

            o_sb = work.tile([128, 128], F32, tag="o")
            nc.vector.tensor_scalar(
                out=o_sb, in0=po_ps, scalar1=rs_sb, scalar2=None,
                op0=mybir.AluOpType.mult,
            )
            nc.scalar.dma_start(
                out=out_ap[g0 : g0 + GRP].rearrange("r (q e) -> (r q) e", e=D),
                in_=o_sb,
            )


def _build(rows, cc):
    nc = bacc.Bacc(trn_type="TRN2", target_bir_lowering=False, debug=False)
    with tile.TileContext(nc) as tc:
        _emit(nc, tc, rows, cc)
    nc.compile()
    return nc


def _schedule(mask_b):
    """Sort rows by length (descending), deal into slots of N_CORES*GRP.

    Returns (perms, cc): perms[c] is core c's row order (global indices),
    cc[k] = chunks needed by slot k — identical across cores.
    """
    n_real = mask_b.sum(axis=1)
    order = np.argsort(-n_real, kind="stable")
    n_slot = B // (N_CORES * GRP)
    cc = []
    for k in range(n_slot):
        nmax = int(n_real[order[k * N_CORES * GRP]])
        cc.append(max(1, -(-nmax // 128)))
    perms = []
    for c in range(N_CORES):
        idx = np.concatenate(
            [
                order[k * N_CORES * GRP + c * GRP : k * N_CORES * GRP + (c + 1) * GRP]
                for k in range(n_slot)
            ]
        )
        perms.append(idx)
    return perms, tuple(cc)


def host_consts(queries, Wk, log_temperature, Wc, bc, Wv):
    """Fold projections/scales into small host-side constants."""
    queries = np.asarray(queries, np.float64)
    Wk = np.asarray(Wk, np.float64)
    Wc = np.asarray(Wc, np.float64)
    bc = np.asarray(bc, np.float64)
    Wv = np.asarray(Wv, np.float64)
    lt = np.asarray(log_temperature, np.float64)

    scale = D ** -0.5
    inv_t = np.repeat(np.exp(-lt), H)  # [Q]
    s_q = scale * inv_t  # [Q]

    q_eff = queries.reshape(Q, D) + bc.reshape(Q, D)  # [Q, D]
    qk0 = q_eff @ Wk  # [Q, D]
    # maug[c, q, d]: rows 0..C-1 = SC*s_q * (Wc_q^T @ Wk); row C = SC*s_q * qk0
    maug = np.empty((C + 1, Q, D), np.float64)
    for q in range(Q):
        Wc_q = Wc[q * D : (q + 1) * D, :]  # [D(e), C]
        maug[:C, q, :] = (Wc_q.T @ Wk) * (SC * s_q[q])
        maug[C, q, :] = qk0[q] * (SC * s_q[q])

    return dict(
        maug=maug.astype(np.float16),
        wvt=np.ascontiguousarray(Wv.T).astype(np.float16),
        id16=np.eye(128, dtype=np.float16),
        id32=np.eye(128, dtype=np.float32),
    )


def _slopes_q():
    slopes_h = 2.0 ** (-8.0 * (np.arange(H) + 1) / H)
    return np.tile(slopes_h, T)  # [Q]


def make_in_maps(keys, mask, context, consts, rows, n_cores, perms):
    keys16 = np.asarray(keys, np.float32).astype(np.float16)  # [B, S, D]
    # kk[:, :, 0, :] = keys^T ([d, s]); kk[:, :, 1, :] = pass2 tiles
    # ([p, (c d)], s = c*128 + p). One DMA per group covers both.
    kk = np.empty((B, 128, 2, S), np.float16)
    kk[:, :, 0, :] = keys16.transpose(0, 2, 1)
    kk[:, :, 1, :] = (
        keys16.reshape(B, NCH, 128, D).transpose(0, 2, 1, 3).reshape(B, 128, S)
    )
    mask_b = np.asarray(mask).astype(bool)
    ctx = np.asarray(context, np.float32)

    n_real = mask_b.sum(axis=1).astype(np.float64)  # [B]
    slopes = _slopes_q()  # [Q]
    n_grp = rows // GRP

    # m6[:, :, :S] (rhs)  rows 0-3: mask[r] f16; row 4: s values; row 5: ones
    # m6[:, :, S:] (lhsT) rows 0-3: MASK_NEG on the r-th 32-col block;
    #      row 4: SC*slope_q ; row 5: -(MASK_NEG + SC*slope_q*(n_r - 1))
    svals = np.arange(S, dtype=np.float16)
    slope_row = np.tile(SC * slopes, 128 // Q).astype(np.float16)

    in_maps = []
    for i in range(n_cores):
        perm = perms[i]
        mk = mask_b[perm]  # [rows, S]
        nr = n_real[perm]  # [rows]
        m6 = np.zeros((n_grp, 6, S + 128), np.float16)
        m6[:, :GRP, :S] = mk.astype(np.float16).reshape(n_grp, GRP, S)
        m6[:, 4, :S] = svals
        m6[:, 5, :S] = 1.0
        m6[:, 4, S:] = slope_row
        for r in range(GRP):
            m6[:, r, S + 32 * r : S + 32 * (r + 1)] = MASK_NEG
            m6[:, 5, S + 32 * r : S + 32 * (r + 1)] = -(
                MASK_NEG + SC * slopes[None, :] * (nr[r::GRP, None] - 1.0)
            ).astype(np.float16)
        kk_core = np.ascontiguousarray(
            kk[perm].reshape(n_grp, GRP, 128, 2, S).transpose(0, 2, 1, 3, 4)
        )
        in_maps.append(
            dict(
                kk=kk_core,
                ctx=np.ascontiguousarray(ctx[perm]),
                m6=m6,
                **consts,
            )
        )
    return in_maps


_cache = {}


def run(keys, mask, context, queries, Wk, Wv, log_temperature, Wc, bc,
        trace=False, **kw):
    consts = host_consts(queries, Wk, log_temperature, Wc, bc, Wv)
    mask_b = np.asarray(mask).astype(bool)
    perms, cc = _schedule(mask_b)
    key = (ROWS, cc)
    if key not in _cache:
        _cache[key] = _build(ROWS, cc)
    nc = _cache[key]
    in_maps = make_in_maps(keys, mask, context, consts, ROWS, N_CORES, perms)
    res = run_bass_kernel_spmd(nc, in_maps, core_ids=list(range(N_CORES)),
                               trace=trace, **kw)
    out = np.empty((B, Q * D), np.float32)
    for i in range(N_CORES):
        out[perms[i]] = res.results[i]["out"]
    return out.reshape(B, T, H * D), res


def kernel(keys, mask, context, queries, Wk, Wv, log_temperature, Wc, bc):
    out, _ = run(keys, mask, context, queries, Wk, Wv, log_temperature, Wc, bc)
    return out


# revision 22
# speedup vs baseline: 2.4569x; 1.0627x over previous
"""AttentionPool TRN2 kernel.

Problem: B=2048, S=512, D=128, H=4, T=8 (Q = T*H = 32), C=64.
  k = keys @ Wk^T ; v = keys @ Wv^T
  q = q_flat + (ctx @ Wc^T + bc).reshape(B, Q, D)
  attn = (q @ k^T) * scale * inv_t[q] - slopes[q] * games_ago[s]
  out  = softmax_masked(attn) @ v            -> [B, T, H*D]

Structure (v2):
  - Host pre-casts keys to f16 and ships BOTH orientations:
      kt [rows, D, S]        (logits rhs: contract d)
      kp [rows, 128, 4, 128] (pass2 stationary tiles, s = c*128 + p)
    Same DMA bytes as one f32 copy, but no on-device cast and no PE
    key transposes.
  - All additive logit terms (ALiBi slope*s, -slope*(n-1) shift, mask)
    ride a single [6,*] matmul into the logits PSUM: rows 0-3 select
    MASK_NEG*mask[r,s], row 4 adds SC*slope_q*s, row 5 adds the
    per-(r,q) constant -(MASK_NEG + SC*slope_q*(n_r-1)) (host-computed
    from the mask). Softmax then needs no row-max: true logits <= ~2.
  - exp on scalar engine -> f16 weights + f32 row sums in one pass.
  - w^T via f16 PE transposes; pooled = (w @ keys) @ Wv^T in f16.

Sharding: pure data parallel over batch, 256 rows/core on 8 cores.
"""

import sys

if "/opt/trn_rl_repo" not in sys.path:
    sys.path.insert(0, "/opt/trn_rl_repo")

import numpy as np

import concourse.bacc as bacc
import concourse.bass as bass
import concourse.tile as tile
from concourse import mybir
from concourse.bass_utils import run_bass_kernel_spmd

B, S, D, H, T, C = 2048, 512, 128, 4, 8, 64
Q = T * H  # 32
N_CORES = 8
ROWS = B // N_CORES  # 256 rows per core
GRP = 4  # batch rows per group -> 4*32 = 128 partitions
BLK = 128  # rows per block (ctx/QKT staging)
SC = 64.0  # power-of-two prescale keeping f16 operands in normal range
MASK_NEG = 16384.0  # f16-exact; /SC = 256 pushes masked logits below -126

F32 = mybir.dt.float32
F16 = mybir.dt.float16

NCH = S // 128  # 4 s-chunks


def _emit(nc, tc, rows, cc):
    """Emit the per-core program for `rows` batch rows (rows % GRP == 0).

    kk[row] packs both key orientations: [:, 0, :] = keys^T row ([d, s]),
    [:, 1, :] = pass2 tiles ([p, (c d)], s = c*128 + p). m6 packs the
    bias/mask matmul rhs (cols 0..S-1) and lhsT (cols S..S+127).

    cc[g] in 1..4 is the number of 128-wide s-chunks group g touches.
    Rows are host-sorted by length so every skipped chunk is fully
    masked; its softmax terms are exact zeros, so the result is
    identical to the full computation.
    """
    kk_d = nc.declare_dram_parameter(
        "kk", [rows // GRP, 128, GRP, 2, S], F16, isOutput=False
    )
    ctx_d = nc.declare_dram_parameter("ctx", [rows, C], F32, isOutput=False)
    m6_d = nc.declare_dram_parameter("m6", [rows // GRP, 6, S + 128], F16, isOutput=False)
    maug_d = nc.declare_dram_parameter("maug", [C + 1, Q, D], F16, isOutput=False)
    wvt_d = nc.declare_dram_parameter("wvt", [D, D], F16, isOutput=False)
    id16_d = nc.declare_dram_parameter("id16", [128, 128], F16, isOutput=False)
    id32_d = nc.declare_dram_parameter("id32", [128, 128], F32, isOutput=False)
    out_d = nc.declare_dram_parameter("out", [rows, Q * D], F32, isOutput=True)

    kk_ap = kk_d.ap()
    ctx_ap = ctx_d.ap()
    out_ap = out_d.ap()

    n_blk = (rows + BLK - 1) // BLK

    import contextlib

    with contextlib.ExitStack() as ctx:
        singles = ctx.enter_context(tc.tile_pool(name="singles", bufs=1))
        kpool = ctx.enter_context(tc.tile_pool(name="kpool", bufs=5))
        mpool = ctx.enter_context(tc.tile_pool(name="mpool", bufs=6))
        blkpool = ctx.enter_context(tc.tile_pool(name="blkpool", bufs=2))
        qktpool = ctx.enter_context(tc.tile_pool(name="qktpool", bufs=2))
        work = ctx.enter_context(tc.tile_pool(name="work", bufs=3))
        small = ctx.enter_context(tc.tile_pool(name="small", bufs=4))
        ps = ctx.enter_context(tc.tile_pool(name="ps", bufs=1, space="PSUM"))

        # ---- constants (loaded once) ----
        maug_sb = singles.tile([C + 1, Q, D], F16)
        nc.sync.dma_start(out=maug_sb, in_=maug_d.ap())
        wvt_sb = singles.tile([D, D], F16)
        nc.sync.dma_start(out=wvt_sb, in_=wvt_d.ap())
        id16_sb = singles.tile([128, 128], F16)
        nc.sync.dma_start(out=id16_sb, in_=id16_d.ap())
        id32_sb = singles.tile([128, 128], F32)
        nc.sync.dma_start(out=id32_sb, in_=id32_d.ap())

        # ---- prologue: conditioned queries qk'^T for every block ----
        qkt_blocks = []
        for blk in range(n_blk):
            r0 = blk * BLK
            bn = min(BLK, rows - r0)
            assert bn % GRP == 0

            ctx_sb = blkpool.tile([BLK, C], F32, tag="ctx")
            nc.sync.dma_start(out=ctx_sb[:bn], in_=ctx_ap[r0 : r0 + bn])
            ctxt_ps = ps.tile([C, BLK], F32, tag="smallf32", bufs=1)
            nc.tensor.transpose(ctxt_ps[:, :bn], ctx_sb[:bn], id32_sb[:bn, :bn])
            ctxt_sb = blkpool.tile([C + 1, BLK], F16, tag="ctxt")
            nc.vector.tensor_copy(out=ctxt_sb[:C, :bn], in_=ctxt_ps[:, :bn])
            nc.vector.memset(ctxt_sb[C : C + 1, :bn], 1.0)

            # qk'^T for the block: [D, bn, Q] f16 (prescaled by SC*scale*inv_t)
            qkt_sb = qktpool.tile([D, BLK, Q], F16, tag="qkt")
            for q in range(Q):
                qkt_ps = ps.tile([D, BLK], F32, tag="smallf32", bufs=1)
                nc.tensor.matmul(
                    qkt_ps[:, :bn], maug_sb[:, q, :], ctxt_sb[:, :bn],
                    start=True, stop=True,
                )
                nc.vector.tensor_copy(out=qkt_sb[:, :bn, q], in_=qkt_ps[:, :bn])
            qkt_blocks.append(qkt_sb)

        n_grp_total = rows // GRP
        PF = 2  # software prefetch distance (groups)
        staged = {}

        def _load_group(g):
            if g >= n_grp_total or g in staged:
                return
            sl = cc[g] * 128
            kkg = kpool.tile([128, GRP, 2, S], F16, tag="kk", name=f"kk_{g}")
            nc.sync.dma_start(
                out=kkg[:, :, :, :sl],
                in_=kk_ap[g, :, :, :, :sl],
            )
            m6 = mpool.tile([6, S + 128], F16, tag="m6", name=f"m6_{g}")
            nc.gpsimd.dma_start(out=m6, in_=m6_d.ap()[g])
            staged[g] = (kkg, m6)

        for g in range(PF):
            _load_group(g)

        # Two-stage software pipeline: iteration g runs logits/mask/exp
        # for group g while the PE finishes wT/pass2/pooled for group
        # g-1. The PE never waits on the scalar EXP of the same group.
        prev = None
        for g in range(n_grp_total + 1):
            if g < n_grp_total:
                g0 = g * GRP  # absolute row of this group
                qkt_sb = qkt_blocks[g0 // BLK]
                gl = g0 % BLK  # row offset inside the block
                _load_group(g + PF)
                kkg, m6 = staged.pop(g)
                ncg = cc[g]
                sl = ncg * 128

                # ---- logits psum: qk' . k^T  (+ bias/mask matmul) ----
                lg_ps = ps.tile([128, S], F32, tag="logits", bufs=2)
                for r in range(GRP):
                    nc.tensor.matmul(
                        lg_ps[32 * r : 32 * (r + 1), :sl],
                        qkt_sb[:, gl + r, :],
                        kkg[:, r, 0, :sl],
                        start=True, stop=False,
                        tile_position=(0, 32 * r),
                        skip_group_check=True,
                    )
                nc.tensor.matmul(
                    lg_ps[:, :sl], m6[:, S : S + 128], m6[:, :sl],
                    start=False, stop=True,
                    skip_group_check=True,
                )

                # ---- softmax (no row max needed: true logits <= ~2) ----
                e16 = work.tile([128, S], F16, tag="e16")
                sum_sb = small.tile([128, 1], F32, tag="sum")
                nc.scalar.activation(
                    out=e16[:, :sl], in_=lg_ps[:, :sl],
                    func=mybir.ActivationFunctionType.Exp,
                    scale=1.0 / SC, accum_out=sum_sb,
                )
                cur = (g0, kkg, e16, sum_sb, ncg)
            else:
                cur = None

            if prev is not None:
                pg0, pkkg, pe16, psum_sb, pncg = prev

                # ---- w^T: [s_in_chunk, c, rq] f16 via PE transposes ----
                wt_ps = ps.tile([128, NCH, 128], F16, tag="wtps", bufs=2)
                for c in range(pncg):
                    nc.tensor.transpose(
                        wt_ps[:, c, :], pe16[:, c * 128 : (c + 1) * 128], id16_sb
                    )
                wt16 = work.tile([128, NCH, 128], F16, tag="wt")
                nc.vector.tensor_copy(out=wt16[:, :pncg], in_=wt_ps[:, :pncg])

                # ---- pass 2: pk[d, rq] = sum_s keys[s,d] * w[s,rq] ----
                pk_ps = ps.tile([128, 128], F32, tag="pk", bufs=2)
                for r in range(GRP):
                    for c in range(pncg):
                        nc.tensor.matmul(
                            pk_ps[:, 32 * r : 32 * (r + 1)],
                            pkkg[:, r, 1, c * 128 : (c + 1) * 128],
                            wt16[:, c, 32 * r : 32 * (r + 1)],
                            start=(c == 0), stop=(c == pncg - 1),
                            skip_group_check=True,
                        )
                pkt16 = work.tile([128, 128], F16, tag="pkt")
                nc.vector.tensor_copy(out=pkt16, in_=pk_ps)

                # ---- pooled[rq, e] = pk^T @ Wv^T, scaled by 1/rowsum ----
                po_ps = ps.tile([128, 128], F32, tag="po", bufs=1)
                nc.tensor.matmul(po_ps, pkt16, wvt_sb, start=True, stop=True)

                rs_sb = small.tile([128, 1], F32, tag="rs")
                nc.vector.reciprocal(rs_sb, psum_sb)
                o_sb = work.tile([128, 128], F32, tag="o")
                nc.vector.tensor_scalar(
                    out=o_sb, in0=po_ps, scalar1=rs_sb, scalar2=None,
                    op0=mybir.AluOpType.mult,
                )
                nc.gpsimd.dma_start(
                    out=out_ap[pg0 : pg0 + GRP].rearrange("r (q e) -> (r q) e", e=D),
                    in_=o_sb,
                )

            prev = cur


def _build(rows, cc):
    nc = bacc.Bacc(trn_type="TRN2", target_bir_lowering=False, debug=False)
    with tile.TileContext(nc) as tc:
        _emit(nc, tc, rows, cc)
    nc.compile()
    return nc


def _schedule(mask_b):
    """Sort rows by length (descending), deal into slots of N_CORES*GRP.

    Returns (perms, cc): perms[c] is core c's row order (global indices),
    cc[k] = chunks needed by slot k — identical across cores.
    """
    n_real = mask_b.sum(axis=1)
    order = np.argsort(-n_real, kind="stable")
    n_slot = B // (N_CORES * GRP)
    cc = []
    for k in range(n_slot):
        nmax = int(n_real[order[k * N_CORES * GRP]])
        cc.append(max(1, -(-nmax // 128)))
    perms = []
    for c in range(N_CORES):
        idx = np.concatenate(
            [
                order[k * N_CORES * GRP + c * GRP : k * N_CORES * GRP + (c + 1) * GRP]
                for k in range(n_slot)
            ]
        )
        perms.append(idx)
    return perms, tuple(cc)


def host_consts(queries, Wk, log_temperature, Wc, bc, Wv):
    """Fold projections/scales into small host-side constants."""
    queries = np.asarray(queries, np.float64)
    Wk = np.asarray(Wk, np.float64)
    Wc = np.asarray(Wc, np.float64)
    bc = np.asarray(bc, np.float64)
    Wv = np.asarray(Wv, np.float64)
    lt = np.asarray(log_temperature, np.float64)

    scale = D ** -0.5
    inv_t = np.repeat(np.exp(-lt), H)  # [Q]
    s_q = scale * inv_t  # [Q]

    q_eff = queries.reshape(Q, D) + bc.reshape(Q, D)  # [Q, D]
    qk0 = q_eff @ Wk  # [Q, D]
    # maug[c, q, d]: rows 0..C-1 = SC*s_q * (Wc_q^T @ Wk); row C = SC*s_q * qk0
    maug = np.empty((C + 1, Q, D), np.float64)
    for q in range(Q):
        Wc_q = Wc[q * D : (q + 1) * D, :]  # [D(e), C]
        maug[:C, q, :] = (Wc_q.T @ Wk) * (SC * s_q[q])
        maug[C, q, :] = qk0[q] * (SC * s_q[q])

    return dict(
        maug=maug.astype(np.float16),
        wvt=np.ascontiguousarray(Wv.T).astype(np.float16),
        id16=np.eye(128, dtype=np.float16),
        id32=np.eye(128, dtype=np.float32),
    )


def _slopes_q():
    slopes_h = 2.0 ** (-8.0 * (np.arange(H) + 1) / H)
    return np.tile(slopes_h, T)  # [Q]


def make_in_maps(keys, mask, context, consts, rows, n_cores, perms):
    keys16 = np.asarray(keys, np.float32).astype(np.float16)  # [B, S, D]
    # kk[:, :, 0, :] = keys^T ([d, s]); kk[:, :, 1, :] = pass2 tiles
    # ([p, (c d)], s = c*128 + p). One DMA per group covers both.
    kk = np.empty((B, 128, 2, S), np.float16)
    kk[:, :, 0, :] = keys16.transpose(0, 2, 1)
    kk[:, :, 1, :] = (
        keys16.reshape(B, NCH, 128, D).transpose(0, 2, 1, 3).reshape(B, 128, S)
    )
    mask_b = np.asarray(mask).astype(bool)
    ctx = np.asarray(context, np.float32)

    n_real = mask_b.sum(axis=1).astype(np.float64)  # [B]
    slopes = _slopes_q()  # [Q]
    n_grp = rows // GRP

    # m6[:, :, :S] (rhs)  rows 0-3: mask[r] f16; row 4: s values; row 5: ones
    # m6[:, :, S:] (lhsT) rows 0-3: MASK_NEG on the r-th 32-col block;
    #      row 4: SC*slope_q ; row 5: -(MASK_NEG + SC*slope_q*(n_r - 1))
    svals = np.arange(S, dtype=np.float16)
    slope_row = np.tile(SC * slopes, 128 // Q).astype(np.float16)

    in_maps = []
    for i in range(n_cores):
        perm = perms[i]
        mk = mask_b[perm]  # [rows, S]
        nr = n_real[perm]  # [rows]
        m6 = np.zeros((n_grp, 6, S + 128), np.float16)
        m6[:, :GRP, :S] = mk.astype(np.float16).reshape(n_grp, GRP, S)
        m6[:, 4, :S] = svals
        m6[:, 5, :S] = 1.0
        m6[:, 4, S:] = slope_row
        for r in range(GRP):
            m6[:, r, S + 32 * r : S + 32 * (r + 1)] = MASK_NEG
            m6[:, 5, S + 32 * r : S + 32 * (r + 1)] = -(
                MASK_NEG + SC * slopes[None, :] * (nr[r::GRP, None] - 1.0)
            ).astype(np.float16)
        kk_core = np.ascontiguousarray(
            kk[perm].reshape(n_grp, GRP, 128, 2, S).transpose(0, 2, 1, 3, 4)
        )
        in_maps.append(
            dict(
                kk=kk_core,
                ctx=np.ascontiguousarray(ctx[perm]),
                m6=m6,
                **consts,
            )
        )
    return in_maps


_cache = {}


def run(keys, mask, context, queries, Wk, Wv, log_temperature, Wc, bc,
        trace=False, **kw):
    consts = host_consts(queries, Wk, log_temperature, Wc, bc, Wv)
    mask_b = np.asarray(mask).astype(bool)
    perms, cc = _schedule(mask_b)
    key = (ROWS, cc)
    if key not in _cache:
        _cache[key] = _build(ROWS, cc)
    nc = _cache[key]
    in_maps = make_in_maps(keys, mask, context, consts, ROWS, N_CORES, perms)
    res = run_bass_kernel_spmd(nc, in_maps, core_ids=list(range(N_CORES)),
                               trace=trace, **kw)
    out = np.empty((B, Q * D), np.float32)
    for i in range(N_CORES):
        out[perms[i]] = res.results[i]["out"]
    return out.reshape(B, T, H * D), res


def kernel(keys, mask, context, queries, Wk, Wv, log_temperature, Wc, bc):
    out, _ = run(keys, mask, context, queries, Wk, Wv, log_temperature, Wc, bc)
    return out
